# revision 14
# baseline (speedup 1.0000x reference)
"""Trainium2 Bass kernel for ContextEncodingTransformer (layer_id==1 path).

Data-parallel over BT across 8 NeuronCores: core i handles image batches
8i..8i+8 (= output rows 96i..96i+96).

Per-core dataflow (all matmuls in float32r, fp32 PSUM accumulate):
  roi conv1x1   : conv_roi[c,(r,p)] = w_ds1T.T @ roi_t          (K=d, 8 chunks)
  emb           : emb_rc[r,o]      += conv_roi[:,:,p].T @ w_embT[:,p,:]
  emb^T         : PE transpose -> embT[c,r]  (score stationary)
  per image b:
    img conv1x1 : img[c,s] = w_ds2T.T @ raw                      (K=C, 6 chunks)
    combo       : img_chunk.T @ [I_128 | embT_b] -> [imgT tile | a^T cols]
                  (one matmul transposes img AND computes scores)
    exp         : E^T = exp(a^T - SHIFT)   (global shift; softmax-invariant)
    ctx         : ctx[n, 0:256]+rowsum = E^T.T @ [imgT | ones]   (K=s, 29 chunks)
    scale       : ctx *= 1/rowsum
  tail          : LN1(ctx+emb) -> x; x^T; FFN (relu) in [c,r]; (ffn)^T;
                  LN2(x+ffn) -> out[96, 256]
"""

import os
import sys
import types

import numpy as np

sys.path.insert(0, "/opt/trn_rl_repo")

import concourse.bass as bass
import concourse.tile as tile
from concourse import bacc, mybir
from concourse.bass_utils import run_bass_kernel_spmd
from concourse.masks import make_identity

F32 = mybir.dt.float32
F32R = mybir.dt.float32r

N_CORES = 8
NFC = 256
D = 1024
KK = 25          # 5x5 kernel positions
NROI = 12        # rois per image
B = 8            # images per core
R = 96           # rows per core (B * NROI)
S = 3600         # 45*80 spatial
CIMG = 768
SHIFT = 60.0     # global softmax shift (max logit ~85; see notes)
EPS = 1e-5

NK = 29          # s-chunks of 128 (28*128 + 16)
SK_LAST = S - 128 * (NK - 1)

LAST_RESULTS = None  # BassKernelResults of the most recent run (for test.py)


def _r(x):
    return x


def build_bass():
    nc = bacc.Bacc("TRN2", target_bir_lowering=False, debug=False,
                   num_devices=N_CORES)

    # ---- DRAM I/O (per-core shard) ----
    roi_d = nc.dram_tensor("roi_t", [D, R * KK], F32R, kind="ExternalInput").ap()
    img_d = nc.dram_tensor("img_raw", [B, CIMG, S], F32R, kind="ExternalInput").ap()
    w1_d = nc.dram_tensor("w_ds1t", [D, NFC], F32R, kind="ExternalInput").ap()
    w2_d = nc.dram_tensor("w_ds2t", [CIMG, NFC], F32R, kind="ExternalInput").ap()
    we_d = nc.dram_tensor("w_embt", [NFC, KK * NFC], F32R, kind="ExternalInput").ap()
    b1_d = nc.dram_tensor("b_ds1", [NFC], F32, kind="ExternalInput").ap()
    b2_d = nc.dram_tensor("b_ds2", [NFC], F32, kind="ExternalInput").ap()
    be_d = nc.dram_tensor("b_emb", [NFC], F32, kind="ExternalInput").ap()
    fw1_d = nc.dram_tensor("ffn_w1t", [NFC, NFC], F32R, kind="ExternalInput").ap()
    fb1_d = nc.dram_tensor("ffn_b1", [NFC], F32, kind="ExternalInput").ap()
    fw2_d = nc.dram_tensor("ffn_w2t", [NFC, NFC], F32R, kind="ExternalInput").ap()
    fb2_d = nc.dram_tensor("ffn_b2", [NFC], F32, kind="ExternalInput").ap()
    g1_d = nc.dram_tensor("g1", [NFC], F32, kind="ExternalInput").ap()
    be1_d = nc.dram_tensor("be1", [NFC], F32, kind="ExternalInput").ap()
    g2_d = nc.dram_tensor("g2", [NFC], F32, kind="ExternalInput").ap()
    be2_d = nc.dram_tensor("be2", [NFC], F32, kind="ExternalInput").ap()
    out_d = nc.dram_tensor("out", [R, NFC], F32, kind="ExternalOutput").ap()

    with tile.TileContext(nc) as tc:
        _body(nc, tc, roi_d, img_d, w1_d, w2_d, we_d, b1_d, b2_d, be_d,
              fw1_d, fb1_d, fw2_d, fb2_d, g1_d, be1_d, g2_d, be2_d, out_d)

    nc.compile()
    return nc


def _body(nc, tc, roi_d, img_d, w1_d, w2_d, we_d, b1_d, b2_d, be_d,
          fw1_d, fb1_d, fw2_d, fb2_d, g1_d, be1_d, g2_d, be2_d, out_d):
    from contextlib import ExitStack

    # ---------- persistent small tiles ----------
    persist_cm = tc.tile_pool(name="persist", bufs=1)
    persist = persist_cm.__enter__()

    ident = persist.tile([128, 128], F32)
    make_identity(nc, ident[:])

    # per-partition biases [128, 2] (col = 128-half index)
    b2_sb = persist.tile([128, 2], F32)
    nc.sync.dma_start(out=b2_sb[:], in_=b2_d.rearrange("(c p) -> p c", p=128))
    fb1_sb = persist.tile([128, 2], F32)
    nc.sync.dma_start(out=fb1_sb[:], in_=fb1_d.rearrange("(c p) -> p c", p=128))
    fb2_sb = persist.tile([128, 2], F32)
    nc.sync.dma_start(out=fb2_sb[:], in_=fb2_d.rearrange("(c p) -> p c", p=128))
    b1_sb = persist.tile([128, 2], F32)
    nc.sync.dma_start(out=b1_sb[:], in_=b1_d.rearrange("(c p) -> p c", p=128))

    # free-dim (broadcast over rows) vectors [R, 256]
    def bcast(src):
        t = persist.tile([R, NFC], F32)
        nc.sync.dma_start(out=t[:], in_=src[None, :].to_broadcast((R, NFC)))
        return t

    bemb_bc = bcast(be_d)
    g1_bc = bcast(g1_d)
    be1_bc = bcast(be1_d)
    g2_bc = bcast(g2_d)
    be2_bc = bcast(be2_d)

    ones58 = persist.tile([128, 58], F32)
    nc.vector.memset(ones58[:], 1.0)
    nshift_sb = persist.tile([128, 1], F32)
    nc.vector.memset(nshift_sb[:], -SHIFT)
    eps_sb = persist.tile([128, 1], F32)
    nc.vector.memset(eps_sb[:], EPS)

    # outputs of roi phase that persist through the image loop
    emb_rc = persist.tile([R, NFC], F32)      # emb in [row, o]
    embT = persist.tile([128, 2, R], F32)     # emb^T  [c_part, c_half, row]
    ctx_sb = persist.tile([R, NFC], F32)      # attention context, packed rows

    # ==================== ROI phase ====================
    with ExitStack() as roi_ctx:
        rp = roi_ctx.enter_context(tc.tile_pool(name="roi", bufs=1))
        rps = roi_ctx.enter_context(
            tc.tile_pool(name="roi_ps", bufs=4, space="PSUM"))
        rps1 = roi_ctx.enter_context(
            tc.tile_pool(name="roi_ps1", bufs=1, space="PSUM"))
        rps2 = roi_ctx.enter_context(
            tc.tile_pool(name="roi_ps2", bufs=2, space="PSUM"))

        roisb = rp.tile([128, 8, R * KK], F32R)  # [d_part, d_chunk, (r p)]
        nc.sync.dma_start(out=roisb[:],
                          in_=roi_d.rearrange("(k p) f -> p k f", p=128))
        w1sb = rp.tile([128, 8, NFC], F32R)
        nc.sync.dma_start(out=w1sb[:],
                          in_=w1_d.rearrange("(k p) o -> p k o", p=128))
        wesb = rp.tile([128, 2, KK, NFC], F32R)  # [c_part, c_chunk, p, o]
        nc.sync.dma_start(out=wesb[:],
                          in_=we_d.rearrange("(k p) f -> p k f", p=128))

        conv_roi = rp.tile([128, 2, R, KK], F32R)  # [c_part, c_half, r, p]

        # conv1x1 on roi: 2 halves x 5 free-chunks of 480, accumulate 8 d-chunks
        for ch in range(2):
            for f in range(5):
                ps = rps.tile([128, 480], F32, tag="roi_conv")
                for dk in range(8):
                    nc.tensor.matmul(
                        ps[:],
                        _r(w1sb[:, dk, ch * 128:(ch + 1) * 128]),
                        _r(roisb[:, dk, f * 480:(f + 1) * 480]),
                        start=(dk == 0), stop=(dk == 7))
                nc.scalar.activation(
                    out=conv_roi[:, ch].rearrange("p r q -> p (r q)")[
                        :, f * 480:(f + 1) * 480],
                    in_=ps[:],
                    func=mybir.ActivationFunctionType.Identity,
                    bias=b1_sb[:, ch:ch + 1], scale=1.0)

        # emb[r, o] += conv_roi[:, ck, :, p].T @ w_embT[:, ck, p, :]
        eps_ps = rps1.tile([R, NFC], F32, tag="emb")
        for ck in range(2):
            for p in range(KK):
                nc.tensor.matmul(
                    eps_ps[:],
                    _r(conv_roi[:, ck, :, p]),
                    _r(wesb[:, ck, p, :]),
                    start=(ck == 0 and p == 0), stop=(ck == 1 and p == KK - 1))
        nc.vector.tensor_add(emb_rc[:], eps_ps[:], bemb_bc[:])

        # emb^T via PE transpose (2 tiles of [96,128] -> [128,96])
        for ch in range(2):
            tp = rps2.tile([128, R], F32, tag="embt")
            nc.tensor.transpose(tp[:], emb_rc[:, ch * 128:(ch + 1) * 128],
                                ident[:R, :R])
            nc.vector.tensor_copy(embT[:, ch, :], tp[:])

    # ==================== image loop ====================
    with ExitStack() as img_ctx:
        ip_raw = img_ctx.enter_context(tc.tile_pool(name="raw", bufs=2))
        ip_img = img_ctx.enter_context(tc.tile_pool(name="img", bufs=2))
        ip_imgt = img_ctx.enter_context(tc.tile_pool(name="imgt", bufs=2))
        ip_small = img_ctx.enter_context(tc.tile_pool(name="ismall", bufs=3))
        ip_w = img_ctx.enter_context(tc.tile_pool(name="iw", bufs=1))
        ps_conv = img_ctx.enter_context(
            tc.tile_pool(name="ps_conv", bufs=5, space="PSUM"))
        ps_combo = img_ctx.enter_context(
            tc.tile_pool(name="ps_combo", bufs=2, space="PSUM"))
        ps_ctx = img_ctx.enter_context(
            tc.tile_pool(name="ps_ctx", bufs=1, space="PSUM"))

        w2sb = ip_w.tile([128, 6, NFC], F32R)
        nc.sync.dma_start(out=w2sb[:],
                          in_=w2_d.rearrange("(k p) o -> p k o", p=128))

        # score rhs: [identity_128 | embT column block for batch b]
        rhs_sc = [ip_w.tile([128, 140], F32R, tag=f"rhs_sc{ch}",
                            name=f"rhs_sc{ch}")
                  for ch in range(2)]
        for ch in range(2):
            nc.vector.tensor_copy(rhs_sc[ch][:, 0:128], ident[:])

        for b in range(B):
            # batch-b score columns of the combo rhs
            for ch in range(2):
                nc.vector.tensor_copy(
                    rhs_sc[ch][:, 128:140],
                    embT[:, ch, b * NROI:(b + 1) * NROI])

            # img conv: 4 s-tiles of 900
            img_sb = ip_img.tile([128, 2, S], F32R, tag="img")
            for st in range(4):
                raw = ip_raw.tile([128, 6, 900], F32R, tag="raw")
                nc.sync.dma_start(
                    out=raw[:],
                    in_=img_d[b].rearrange("(k p) s -> p k s", p=128)[
                        :, :, st * 900:(st + 1) * 900])
                for ch in range(2):
                    for f2 in range(2):
                        ps = ps_conv.tile([128, 450], F32, tag="conv")
                        for Ck in range(6):
                            nc.tensor.matmul(
                                ps[:],
                                _r(w2sb[:, Ck, ch * 128:(ch + 1) * 128]),
                                _r(raw[:, Ck, f2 * 450:(f2 + 1) * 450]),
                                start=(Ck == 0), stop=(Ck == 5))
                        nc.scalar.activation(
                            out=img_sb[:, ch, st * 900 + f2 * 450:
                                       st * 900 + (f2 + 1) * 450],
                            in_=ps[:],
                            func=mybir.ActivationFunctionType.Identity,
                            bias=b2_sb[:, ch:ch + 1], scale=1.0)

            # combo: transpose img + score columns; then exp -> E^T
            imgt = ip_imgt.tile([128, NK, NFC + 2], F32R, tag="imgt")
            et = ip_small.tile([128, NK, NROI], F32R, tag="et")
            nc.vector.tensor_copy(  # ones columns (rowsum) via F32R-rounding copy
                imgt[:, :, NFC:NFC + 2],
                ones58.rearrange("p (k two) -> p k two", two=2))
            for k in range(NK):
                sk = 128 if k < NK - 1 else SK_LAST
                pt = []
                for ch in range(2):
                    ps = ps_combo.tile([128, 140], F32, tag="combo")
                    nc.tensor.matmul(
                        ps[:sk, :],
                        _r(img_sb[:, ch, k * 128:k * 128 + sk]),
                        _r(rhs_sc[ch][:]),
                        start=True, stop=True)
                    nc.vector.tensor_copy(
                        imgt[:sk, k, ch * 128:(ch + 1) * 128], ps[:sk, 0:128])
                    pt.append(ps)
                aT = ip_small.tile([128, NROI], F32, tag="aT")
                nc.vector.tensor_copy(aT[:sk, :], pt[0][:sk, 128:140])
                aT2 = ip_small.tile([128, NROI], F32, tag="aT2")
                nc.vector.tensor_add(aT2[:sk, :], pt[1][:sk, 128:140],
                                     aT[:sk, :])
                nc.scalar.activation(
                    out=et[:sk, k, :], in_=aT2[:sk, :],
                    func=mybir.ActivationFunctionType.Exp,
                    bias=nshift_sb[:sk, :], scale=1.0)

            # ctx: [12, 257] = sum_k E^T[:,k,:].T @ [imgT | ones]
            cps = ps_ctx.tile([NROI, NFC + 2], F32, tag="ctx")
            for k in range(NK):
                sk = 128 if k < NK - 1 else SK_LAST
                nc.tensor.matmul(
                    cps[:], _r(et[:sk, k, :]), _r(imgt[:sk, k, :]),
                    start=(k == 0), stop=(k == NK - 1))
            rinv = ip_small.tile([NROI, 1], F32, tag="rinv")
            nc.vector.reciprocal(rinv[:], cps[:, NFC:NFC + 1])
            ctx_b = ip_small.tile([NROI, NFC], F32, tag="ctx_b")
            nc.vector.tensor_scalar_mul(ctx_b[:], cps[:, 0:NFC], rinv[:])
            nc.sync.dma_start(out=ctx_sb[b * NROI:(b + 1) * NROI, :],
                              in_=ctx_b[:])

    # ==================== tail: LN1 -> FFN -> LN2 ====================
    with ExitStack() as t_ctx:
        tp = t_ctx.enter_context(tc.tile_pool(name="tail", bufs=1))
        tps = t_ctx.enter_context(
            tc.tile_pool(name="tail_ps", bufs=2, space="PSUM"))

        def layernorm(dst, src, g_bc, b_bc):
            st = tp.tile([R, 6], F32, tag="ln_st")
            mv = tp.tile([R, 2], F32, tag="ln_mv")
            nc.vector.bn_stats(out=st[:], in_=src[:])
            nc.vector.bn_aggr(out=mv[:], in_=st[:])
            rstd = tp.tile([R, 1], F32, tag="ln_rstd")
            nc.scalar.activation(out=rstd[:], in_=mv[:, 1:2],
                                 func=mybir.ActivationFunctionType.Sqrt,
                                 bias=eps_sb[:R, :], scale=1.0)
            nc.vector.reciprocal(rstd[:], rstd[:])
            nc.vector.tensor_scalar(
                out=dst[:], in0=src[:], scalar1=mv[:, 0:1], scalar2=rstd[:],
                op0=mybir.AluOpType.subtract, op1=mybir.AluOpType.mult)
            nc.vector.tensor_mul(dst[:], dst[:], g_bc[:])
            nc.vector.tensor_add(dst[:], dst[:], b_bc[:])

        x_in = tp.tile([R, NFC], F32)
        nc.vector.tensor_add(x_in[:], ctx_sb[:], emb_rc[:])
        x = tp.tile([R, NFC], F32)
        layernorm(x, x_in, g1_bc, be1_bc)

        # x^T  [c, r]
        xT = tp.tile([128, 2, R], F32R)
        for ch in range(2):
            ps = tps.tile([128, R], F32, tag="xT")
            nc.tensor.transpose(ps[:], x[:, ch * 128:(ch + 1) * 128],
                                ident[:R, :R])
            nc.vector.tensor_copy(xT[:, ch, :], ps[:])

        fw1 = tp.tile([128, 2, NFC], F32R)
        nc.sync.dma_start(out=fw1[:],
                          in_=fw1_d.rearrange("(k p) o -> p k o", p=128))
        fw2 = tp.tile([128, 2, NFC], F32R)
        nc.sync.dma_start(out=fw2[:],
                          in_=fw2_d.rearrange("(k p) o -> p k o", p=128))

        # h = relu(W1 @ x^T + b1)   in [o, r]
        h = tp.tile([128, 2, R], F32R)
        for ch in range(2):
            ps = tps.tile([128, R], F32, tag="ffn1")
            for ck in range(2):
                nc.tensor.matmul(ps[:],
                                 _r(fw1[:, ck, ch * 128:(ch + 1) * 128]),
                                 _r(xT[:, ck, :]),
                                 start=(ck == 0), stop=(ck == 1))
            nc.scalar.activation(out=h[:, ch, :], in_=ps[:],
                                 func=mybir.ActivationFunctionType.Relu,
                                 bias=fb1_sb[:, ch:ch + 1], scale=1.0)

        # f = W2 @ h + b2   in [o, r], then transpose back to [r, c]
        f_rc = tp.tile([R, NFC], F32)
        for ch in range(2):
            ps = tps.tile([128, R], F32, tag="ffn2")
            for ck in range(2):
                nc.tensor.matmul(ps[:],
                                 _r(fw2[:, ck, ch * 128:(ch + 1) * 128]),
                                 _r(h[:, ck, :]),
                                 start=(ck == 0), stop=(ck == 1))
            fo = tp.tile([128, R], F32, tag="ffn_o")
            nc.scalar.activation(out=fo[:], in_=ps[:],
                                 func=mybir.ActivationFunctionType.Identity,
                                 bias=fb2_sb[:, ch:ch + 1], scale=1.0)
            pst = tps.tile([R, 128], F32, tag="ffn_t")
            nc.tensor.transpose(pst[:], fo[:], ident[:])
            nc.vector.tensor_copy(f_rc[:, ch * 128:(ch + 1) * 128], pst[:])

        x2 = tp.tile([R, NFC], F32)
        nc.vector.tensor_add(x2[:], x[:], f_rc[:])
        out_sb = tp.tile([R, NFC], F32)
        layernorm(out_sb, x2, g2_bc, be2_bc)
        nc.sync.dma_start(out=out_d[:], in_=out_sb[:])


_NC_CACHE = None


def _get_nc():
    global _NC_CACHE
    if _NC_CACHE is None:
        _NC_CACHE = build_bass()
    return _NC_CACHE


def _install_ntff_hook():
    """The image's antenv lacks axon_hooks; register the NTFF profile hook."""
    if "antenv.axon_hooks" in sys.modules:
        return
    try:
        sys.path.insert(0, "/root/.axon_site/trn_agent_boot")
        import trn_boot
        hook = trn_boot._ntff_profile_via_ctypes("/opt/axon/libaxon_pjrt.so")
        m = types.ModuleType("antenv.axon_hooks")
        m.get_axon_ntff_profile_hook = lambda: hook
        sys.modules["antenv.axon_hooks"] = m
    except Exception:
        pass


def prepare_in_maps(roi_feature, image_feature, w_ds1, b_ds1, w_ds2, b_ds2,
                    w_emb, b_emb, g1, be1, ffn_w1, ffn_b1, ffn_w2, ffn_b2,
                    g2, be2, layer_id=1, **_unused):
    roi_feature = np.ascontiguousarray(np.asarray(roi_feature, dtype=np.float32))
    image_feature = np.ascontiguousarray(np.asarray(image_feature, dtype=np.float32))

    f32 = lambda x: np.ascontiguousarray(np.asarray(x, dtype=np.float32))
    w_ds1t = f32(w_ds1).T.copy()                       # [D, NFC]
    w_ds2t = f32(w_ds2).T.copy()                       # [CIMG, NFC]
    w_embt = np.ascontiguousarray(
        f32(w_emb).reshape(NFC, NFC, KK).transpose(1, 2, 0).reshape(
            NFC, KK * NFC))                            # [c, p*o]
    ffn_w1t = f32(ffn_w1).T.copy()
    ffn_w2t = f32(ffn_w2).T.copy()

    shared = {
        "w_ds1t": w_ds1t, "w_ds2t": w_ds2t, "w_embt": w_embt,
        "b_ds1": f32(b_ds1), "b_ds2": f32(b_ds2), "b_emb": f32(b_emb),
        "ffn_w1t": ffn_w1t, "ffn_b1": f32(ffn_b1),
        "ffn_w2t": ffn_w2t, "ffn_b2": f32(ffn_b2),
        "g1": f32(g1), "be1": f32(be1), "g2": f32(g2), "be2": f32(be2),
    }

    roi_r = roi_feature.reshape(N_CORES, R, D, KK)
    img_r = image_feature.reshape(N_CORES, B, CIMG, S)
    in_maps = []
    for i in range(N_CORES):
        roi_t = np.ascontiguousarray(
            roi_r[i].transpose(1, 0, 2)).reshape(D, R * KK)
        in_maps.append({"roi_t": roi_t,
                        "img_raw": np.ascontiguousarray(img_r[i]),
                        **shared})
    return in_maps


def kernel(**inputs):
    global LAST_RESULTS
    in_maps = prepare_in_maps(**inputs)
    nc = _get_nc()
    trace = os.environ.get("BASS_KERNEL_TRACE", "0") == "1"
    if trace:
        _install_ntff_hook()
    LAST_RESULTS = run_bass_kernel_spmd(
        nc, in_maps, list(range(N_CORES)), trace=trace)
    out = np.concatenate([LAST_RESULTS.results[i]["out"]
                          for i in range(N_CORES)], axis=0)
    return out


# revision 15
# speedup vs baseline: 1.0113x; 1.0113x over previous
"""Trainium2 Bass kernel for ContextEncodingTransformer (layer_id==1 path).

Data-parallel over BT across 8 NeuronCores: core i handles image batches
8i..8i+8 (= output rows 96i..96i+96).

Per-core dataflow (all matmuls in float32r, fp32 PSUM accumulate):
  roi conv1x1   : conv_roi[c,(r,p)] = w_ds1T.T @ roi_t          (K=d, 8 chunks)
  emb           : emb_rc[r,o]      += conv_roi[:,:,p].T @ w_embT[:,p,:]
  emb^T         : PE transpose -> embT[c,r]  (score stationary)
  per image b:
    img conv1x1 : img[c,s] = w_ds2T.T @ raw                      (K=C, 6 chunks)
    combo       : img_chunk.T @ [I_128 | embT_b] -> [imgT tile | a^T cols]
                  (one matmul transposes img AND computes scores)
    exp         : E^T = exp(a^T - SHIFT)   (global shift; softmax-invariant)
    ctx         : ctx[n, 0:256]+rowsum = E^T.T @ [imgT | ones]   (K=s, 29 chunks)
    scale       : ctx *= 1/rowsum
  tail          : LN1(ctx+emb) -> x; x^T; FFN (relu) in [c,r]; (ffn)^T;
                  LN2(x+ffn) -> out[96, 256]
"""

import os
import sys
import types

import numpy as np

sys.path.insert(0, "/opt/trn_rl_repo")

import concourse.bass as bass
import concourse.tile as tile
from concourse import bacc, mybir
from concourse.bass_utils import run_bass_kernel_spmd
from concourse.masks import make_identity

F32 = mybir.dt.float32
F32R = mybir.dt.float32r

N_CORES = 8
NFC = 256
D = 1024
KK = 25          # 5x5 kernel positions
NROI = 12        # rois per image
B = 8            # images per core
R = 96           # rows per core (B * NROI)
S = 3600         # 45*80 spatial
CIMG = 768
SHIFT = 60.0     # global softmax shift (max logit ~85; see notes)
EPS = 1e-5

NK = 29          # s-chunks of 128 (28*128 + 16)
SK_LAST = S - 128 * (NK - 1)

LAST_RESULTS = None  # BassKernelResults of the most recent run (for test.py)


def _r(x):
    return x


def build_bass():
    nc = bacc.Bacc("TRN2", target_bir_lowering=False, debug=False,
                   num_devices=N_CORES)

    # ---- DRAM I/O (per-core shard) ----
    roi_d = nc.dram_tensor("roi_t", [D, R * KK], F32R, kind="ExternalInput").ap()
    img_d = nc.dram_tensor("img_raw", [B, CIMG, S], F32R, kind="ExternalInput").ap()
    w1_d = nc.dram_tensor("w_ds1t", [D, NFC], F32R, kind="ExternalInput").ap()
    w2_d = nc.dram_tensor("w_ds2t", [CIMG, NFC], F32R, kind="ExternalInput").ap()
    we_d = nc.dram_tensor("w_embt", [NFC, KK * NFC], F32R, kind="ExternalInput").ap()
    b1_d = nc.dram_tensor("b_ds1", [NFC], F32, kind="ExternalInput").ap()
    b2_d = nc.dram_tensor("b_ds2", [NFC], F32, kind="ExternalInput").ap()
    be_d = nc.dram_tensor("b_emb", [NFC], F32, kind="ExternalInput").ap()
    fw1_d = nc.dram_tensor("ffn_w1t", [NFC, NFC], F32R, kind="ExternalInput").ap()
    fb1_d = nc.dram_tensor("ffn_b1", [NFC], F32, kind="ExternalInput").ap()
    fw2_d = nc.dram_tensor("ffn_w2t", [NFC, NFC], F32R, kind="ExternalInput").ap()
    fb2_d = nc.dram_tensor("ffn_b2", [NFC], F32, kind="ExternalInput").ap()
    g1_d = nc.dram_tensor("g1", [NFC], F32, kind="ExternalInput").ap()
    be1_d = nc.dram_tensor("be1", [NFC], F32, kind="ExternalInput").ap()
    g2_d = nc.dram_tensor("g2", [NFC], F32, kind="ExternalInput").ap()
    be2_d = nc.dram_tensor("be2", [NFC], F32, kind="ExternalInput").ap()
    out_d = nc.dram_tensor("out", [R, NFC], F32, kind="ExternalOutput").ap()

    with tile.TileContext(nc) as tc:
        _body(nc, tc, roi_d, img_d, w1_d, w2_d, we_d, b1_d, b2_d, be_d,
              fw1_d, fb1_d, fw2_d, fb2_d, g1_d, be1_d, g2_d, be2_d, out_d)

    nc.compile()
    return nc


def _body(nc, tc, roi_d, img_d, w1_d, w2_d, we_d, b1_d, b2_d, be_d,
          fw1_d, fb1_d, fw2_d, fb2_d, g1_d, be1_d, g2_d, be2_d, out_d):
    from contextlib import ExitStack

    # ---------- long-lived pools (persist across phases) ----------
    top = ExitStack()
    persist = top.enter_context(tc.tile_pool(name="persist", bufs=1))
    ip_w = top.enter_context(tc.tile_pool(name="iw", bufs=1))
    ip_raw = top.enter_context(tc.tile_pool(name="raw", bufs=2))

    ident = persist.tile([128, 128], F32)
    make_identity(nc, ident[:])

    # per-partition biases [128, 2] (col = 128-half index)
    b2_sb = persist.tile([128, 2], F32)
    nc.sync.dma_start(out=b2_sb[:], in_=b2_d.rearrange("(c p) -> p c", p=128))
    fb1_sb = persist.tile([128, 2], F32)
    nc.sync.dma_start(out=fb1_sb[:], in_=fb1_d.rearrange("(c p) -> p c", p=128))
    fb2_sb = persist.tile([128, 2], F32)
    nc.sync.dma_start(out=fb2_sb[:], in_=fb2_d.rearrange("(c p) -> p c", p=128))
    b1_sb = persist.tile([128, 2], F32)
    nc.sync.dma_start(out=b1_sb[:], in_=b1_d.rearrange("(c p) -> p c", p=128))

    # free-dim (broadcast over rows) vectors [R, 256]
    def bcast(src):
        t = persist.tile([R, NFC], F32)
        nc.sync.dma_start(out=t[:], in_=src[None, :].to_broadcast((R, NFC)))
        return t

    bemb_bc = bcast(be_d)
    g1_bc = bcast(g1_d)
    be1_bc = bcast(be1_d)
    g2_bc = bcast(g2_d)
    be2_bc = bcast(be2_d)

    ones58 = persist.tile([128, 58], F32)
    nc.vector.memset(ones58[:], 1.0)
    nshift_sb = persist.tile([128, 1], F32)
    nc.vector.memset(nshift_sb[:], -SHIFT)
    eps_sb = persist.tile([128, 1], F32)
    nc.vector.memset(eps_sb[:], EPS)

    # outputs of roi phase that persist through the image loop
    emb_rc = persist.tile([R, NFC], F32)      # emb in [row, o]
    embT = persist.tile([128, 2, R], F32)     # emb^T  [c_part, c_half, row]
    ctx_sb = persist.tile([R, NFC], F32)      # attention context, packed rows

    w2sb = ip_w.tile([128, 6, NFC], F32R)
    nc.sync.dma_start(out=w2sb[:],
                      in_=w2_d.rearrange("(k p) o -> p k o", p=128))

    # score rhs: [identity_128 | embT column block for batch b]
    rhs_sc = [ip_w.tile([128, 140], F32R, tag=f"rhs_sc{ch}",
                        name=f"rhs_sc{ch}")
              for ch in range(2)]
    for ch in range(2):
        nc.vector.tensor_copy(rhs_sc[ch][:, 0:128], ident[:])

    def load_raw(b, st):
        raw = ip_raw.tile([128, 6, 900], F32R, tag="raw", name=f"raw_{b}_{st}")
        nc.sync.dma_start(
            out=raw[:],
            in_=img_d[b].rearrange("(k p) s -> p k s", p=128)[
                :, :, st * 900:(st + 1) * 900])
        return raw

    # prefetch the first two image s-tiles so PE can start the img conv
    # while the roi phase is still loading/running
    raw_pre = [load_raw(0, 0), load_raw(0, 1)]

    # ==================== ROI phase ====================
    with ExitStack() as roi_ctx:
        rp = roi_ctx.enter_context(tc.tile_pool(name="roi", bufs=1))
        rpg = roi_ctx.enter_context(tc.tile_pool(name="roi_pg", bufs=2))
        rps = roi_ctx.enter_context(
            tc.tile_pool(name="roi_ps", bufs=5, space="PSUM"))
        rps1 = roi_ctx.enter_context(
            tc.tile_pool(name="roi_ps1", bufs=1, space="PSUM"))
        rps2 = roi_ctx.enter_context(
            tc.tile_pool(name="roi_ps2", bufs=2, space="PSUM"))

        w1sb = rp.tile([128, 8, NFC], F32R)
        nc.sync.dma_start(out=w1sb[:],
                          in_=w1_d.rearrange("(k p) o -> p k o", p=128))
        # split the 9.8MB roi activation load per d-chunk so the conv can
        # start as soon as chunk 0 lands
        roisb = rp.tile([128, 8, R * KK], F32R)  # [d_part, d_chunk, (r p)]
        roi_r = roi_d.rearrange("(k p) f -> p k f", p=128)
        for dk in range(8):
            nc.sync.dma_start(out=roisb[:, dk, :], in_=roi_r[:, dk, :])

        conv_roi = rp.tile([128, 2, R, KK], F32R)  # [c_part, c_half, r, p]

        # conv1x1 on roi: weights stay loaded across the 5 free-chunks
        for ch in range(2):
            pss = []
            for dk in range(8):
                for f in range(5):
                    if dk == 0:
                        pss.append(rps.tile([128, 480], F32, tag="roi_conv",
                                            name=f"roiconv_{ch}_{f}"))
                    nc.tensor.matmul(
                        pss[f][:],
                        w1sb[:, dk, ch * 128:(ch + 1) * 128],
                        roisb[:, dk, f * 480:(f + 1) * 480],
                        start=(dk == 0), stop=(dk == 7))
            for f in range(5):
                nc.scalar.activation(
                    out=conv_roi[:, ch].rearrange("p r q -> p (r q)")[
                        :, f * 480:(f + 1) * 480],
                    in_=pss[f][:],
                    func=mybir.ActivationFunctionType.Identity,
                    bias=b1_sb[:, ch:ch + 1], scale=1.0)

        # emb[r, o] += conv_roi[:, ck, :, p].T @ w_embT[:, ck, p, :]
        # w_embT streamed in 5 p-groups to cap SBUF
        we_r = we_d.rearrange("(k p) f -> p k f", p=128)
        eps_ps = rps1.tile([R, NFC], F32, tag="emb")
        for pg in range(5):
            wes = rpg.tile([128, 2, 5, NFC], F32R, tag="wesb",
                           name=f"wesb_{pg}")
            nc.sync.dma_start(
                out=wes[:],
                in_=we_r.rearrange("p k (q o) -> p k q o", o=NFC)[
                    :, :, pg * 5:(pg + 1) * 5, :])
            for ck in range(2):
                for pl in range(5):
                    p = pg * 5 + pl
                    nc.tensor.matmul(
                        eps_ps[:],
                        conv_roi[:, ck, :, p],
                        wes[:, ck, pl, :],
                        start=(pg == 0 and ck == 0 and pl == 0),
                        stop=(pg == 4 and ck == 1 and pl == 4))
        nc.vector.tensor_add(emb_rc[:], eps_ps[:], bemb_bc[:])

        # emb^T via PE transpose (2 tiles of [96,128] -> [128,96])
        for ch in range(2):
            tp = rps2.tile([128, R], F32, tag="embt")
            nc.tensor.transpose(tp[:], emb_rc[:, ch * 128:(ch + 1) * 128],
                                ident[:R, :R])
            nc.vector.tensor_copy(embT[:, ch, :], tp[:])

    # ==================== image loop ====================
    with ExitStack() as img_ctx:
        ip_img = img_ctx.enter_context(tc.tile_pool(name="img", bufs=2))
        ip_imgt = img_ctx.enter_context(tc.tile_pool(name="imgt", bufs=1))
        ip_small = img_ctx.enter_context(tc.tile_pool(name="ismall", bufs=2))
        ps_conv = img_ctx.enter_context(
            tc.tile_pool(name="ps_conv", bufs=5, space="PSUM"))
        ps_combo = img_ctx.enter_context(
            tc.tile_pool(name="ps_combo", bufs=2, space="PSUM"))
        ps_ctx = img_ctx.enter_context(
            tc.tile_pool(name="ps_ctx", bufs=1, space="PSUM"))

        for b in range(B):
            # batch-b score columns of the combo rhs
            for ch in range(2):
                nc.vector.tensor_copy(
                    rhs_sc[ch][:, 128:140],
                    embT[:, ch, b * NROI:(b + 1) * NROI])

            # img conv: 4 s-tiles of 900
            img_sb = ip_img.tile([128, 2, S], F32R, tag="img")
            for st in range(4):
                if raw_pre:
                    raw = raw_pre.pop(0)
                else:
                    raw = load_raw(b, st)
                for ch in range(2):
                    for f2 in range(2):
                        ps = ps_conv.tile([128, 450], F32, tag="conv")
                        for Ck in range(6):
                            nc.tensor.matmul(
                                ps[:],
                                w2sb[:, Ck, ch * 128:(ch + 1) * 128],
                                raw[:, Ck, f2 * 450:(f2 + 1) * 450],
                                start=(Ck == 0), stop=(Ck == 5))
                        nc.scalar.activation(
                            out=img_sb[:, ch, st * 900 + f2 * 450:
                                       st * 900 + (f2 + 1) * 450],
                            in_=ps[:],
                            func=mybir.ActivationFunctionType.Identity,
                            bias=b2_sb[:, ch:ch + 1], scale=1.0)

            # combo: transpose img + score columns; then exp -> E^T
            imgt = ip_imgt.tile([128, NK, NFC + 2], F32R, tag="imgt")
            et = ip_small.tile([128, NK, NROI], F32R, tag="et")
            nc.vector.tensor_copy(  # ones columns (rowsum) via rounding copy
                imgt[:, :, NFC:NFC + 2],
                ones58.rearrange("p (k two) -> p k two", two=2))
            for k in range(NK):
                sk = 128 if k < NK - 1 else SK_LAST
                pt = []
                for ch in range(2):
                    ps = ps_combo.tile([128, 140], F32, tag="combo")
                    nc.tensor.matmul(
                        ps[:sk, :],
                        img_sb[:, ch, k * 128:k * 128 + sk],
                        rhs_sc[ch][:],
                        start=True, stop=True)
                    nc.vector.tensor_copy(
                        imgt[:sk, k, ch * 128:(ch + 1) * 128], ps[:sk, 0:128])
                    pt.append(ps)
                aT = ip_small.tile([128, NROI], F32, tag="aT")
                nc.vector.tensor_copy(aT[:sk, :], pt[0][:sk, 128:140])
                aT2 = ip_small.tile([128, NROI], F32, tag="aT2")
                nc.vector.tensor_add(aT2[:sk, :], pt[1][:sk, 128:140],
                                     aT[:sk, :])
                nc.scalar.activation(
                    out=et[:sk, k, :], in_=aT2[:sk, :],
                    func=mybir.ActivationFunctionType.Exp,
                    bias=nshift_sb[:sk, :], scale=1.0)

            # ctx: [12, 258] = sum_k E^T[:,k,:].T @ [imgT | ones]
            cps = ps_ctx.tile([NROI, NFC + 2], F32, tag="ctx")
            for k in range(NK):
                sk = 128 if k < NK - 1 else SK_LAST
                nc.tensor.matmul(
                    cps[:], et[:sk, k, :], imgt[:sk, k, :],
                    start=(k == 0), stop=(k == NK - 1))
            rinv = ip_small.tile([NROI, 1], F32, tag="rinv")
            nc.vector.reciprocal(rinv[:], cps[:, NFC:NFC + 1])
            ctx_b = ip_small.tile([NROI, NFC], F32, tag="ctx_b")
            nc.vector.tensor_scalar_mul(ctx_b[:], cps[:, 0:NFC], rinv[:])
            nc.sync.dma_start(out=ctx_sb[b * NROI:(b + 1) * NROI, :],
                              in_=ctx_b[:])

    # ==================== tail: LN1 -> FFN -> LN2 ====================
    with ExitStack() as t_ctx:
        tp = t_ctx.enter_context(tc.tile_pool(name="tail", bufs=1))
        tps = t_ctx.enter_context(
            tc.tile_pool(name="tail_ps", bufs=2, space="PSUM"))

        def layernorm(dst, src, g_bc, b_bc):
            st = tp.tile([R, 6], F32, tag="ln_st")
            mv = tp.tile([R, 2], F32, tag="ln_mv")
            nc.vector.bn_stats(out=st[:], in_=src[:])
            nc.vector.bn_aggr(out=mv[:], in_=st[:])
            rstd = tp.tile([R, 1], F32, tag="ln_rstd")
            nc.scalar.activation(out=rstd[:], in_=mv[:, 1:2],
                                 func=mybir.ActivationFunctionType.Sqrt,
                                 bias=eps_sb[:R, :], scale=1.0)
            nc.vector.reciprocal(rstd[:], rstd[:])
            nc.vector.tensor_scalar(
                out=dst[:], in0=src[:], scalar1=mv[:, 0:1], scalar2=rstd[:],
                op0=mybir.AluOpType.subtract, op1=mybir.AluOpType.mult)
            nc.vector.tensor_mul(dst[:], dst[:], g_bc[:])
            nc.vector.tensor_add(dst[:], dst[:], b_bc[:])

        x_in = tp.tile([R, NFC], F32)
        nc.vector.tensor_add(x_in[:], ctx_sb[:], emb_rc[:])
        x = tp.tile([R, NFC], F32)
        layernorm(x, x_in, g1_bc, be1_bc)

        # x^T  [c, r]
        xT = tp.tile([128, 2, R], F32R)
        for ch in range(2):
            ps = tps.tile([128, R], F32, tag="xT")
            nc.tensor.transpose(ps[:], x[:, ch * 128:(ch + 1) * 128],
                                ident[:R, :R])
            nc.vector.tensor_copy(xT[:, ch, :], ps[:])

        fw1 = tp.tile([128, 2, NFC], F32R)
        nc.sync.dma_start(out=fw1[:],
                          in_=fw1_d.rearrange("(k p) o -> p k o", p=128))
        fw2 = tp.tile([128, 2, NFC], F32R)
        nc.sync.dma_start(out=fw2[:],
                          in_=fw2_d.rearrange("(k p) o -> p k o", p=128))

        # h = relu(W1 @ x^T + b1)   in [o, r]
        h = tp.tile([128, 2, R], F32R)
        for ch in range(2):
            ps = tps.tile([128, R], F32, tag="ffn1")
            for ck in range(2):
                nc.tensor.matmul(ps[:],
                                 fw1[:, ck, ch * 128:(ch + 1) * 128],
                                 xT[:, ck, :],
                                 start=(ck == 0), stop=(ck == 1))
            nc.scalar.activation(out=h[:, ch, :], in_=ps[:],
                                 func=mybir.ActivationFunctionType.Relu,
                                 bias=fb1_sb[:, ch:ch + 1], scale=1.0)

        # f = W2 @ h + b2   in [o, r], then transpose back to [r, c]
        f_rc = tp.tile([R, NFC], F32)
        for ch in range(2):
            ps = tps.tile([128, R], F32, tag="ffn2")
            for ck in range(2):
                nc.tensor.matmul(ps[:],
                                 fw2[:, ck, ch * 128:(ch + 1) * 128],
                                 h[:, ck, :],
                                 start=(ck == 0), stop=(ck == 1))
            fo = tp.tile([128, R], F32, tag="ffn_o")
            nc.scalar.activation(out=fo[:], in_=ps[:],
                                 func=mybir.ActivationFunctionType.Identity,
                                 bias=fb2_sb[:, ch:ch + 1], scale=1.0)
            pst = tps.tile([R, 128], F32, tag="ffn_t")
            nc.tensor.transpose(pst[:], fo[:], ident[:])
            nc.vector.tensor_copy(f_rc[:, ch * 128:(ch + 1) * 128], pst[:])

        x2 = tp.tile([R, NFC], F32)
        nc.vector.tensor_add(x2[:], x[:], f_rc[:])
        out_sb = tp.tile([R, NFC], F32)
        layernorm(out_sb, x2, g2_bc, be2_bc)
        nc.sync.dma_start(out=out_d[:], in_=out_sb[:])

    top.close()


_NC_CACHE = None


def _get_nc():
    global _NC_CACHE
    if _NC_CACHE is None:
        _NC_CACHE = build_bass()
    return _NC_CACHE


def _install_ntff_hook():
    """The image's antenv lacks axon_hooks; register the NTFF profile hook."""
    if "antenv.axon_hooks" in sys.modules:
        return
    try:
        sys.path.insert(0, "/root/.axon_site/trn_agent_boot")
        import trn_boot
        hook = trn_boot._ntff_profile_via_ctypes("/opt/axon/libaxon_pjrt.so")
        m = types.ModuleType("antenv.axon_hooks")
        m.get_axon_ntff_profile_hook = lambda: hook
        sys.modules["antenv.axon_hooks"] = m
    except Exception:
        pass


def prepare_in_maps(roi_feature, image_feature, w_ds1, b_ds1, w_ds2, b_ds2,
                    w_emb, b_emb, g1, be1, ffn_w1, ffn_b1, ffn_w2, ffn_b2,
                    g2, be2, layer_id=1, **_unused):
    roi_feature = np.ascontiguousarray(np.asarray(roi_feature, dtype=np.float32))
    image_feature = np.ascontiguousarray(np.asarray(image_feature, dtype=np.float32))

    f32 = lambda x: np.ascontiguousarray(np.asarray(x, dtype=np.float32))
    w_ds1t = f32(w_ds1).T.copy()                       # [D, NFC]
    w_ds2t = f32(w_ds2).T.copy()                       # [CIMG, NFC]
    w_embt = np.ascontiguousarray(
        f32(w_emb).reshape(NFC, NFC, KK).transpose(1, 2, 0).reshape(
            NFC, KK * NFC))                            # [c, p*o]
    ffn_w1t = f32(ffn_w1).T.copy()
    ffn_w2t = f32(ffn_w2).T.copy()

    shared = {
        "w_ds1t": w_ds1t, "w_ds2t": w_ds2t, "w_embt": w_embt,
        "b_ds1": f32(b_ds1), "b_ds2": f32(b_ds2), "b_emb": f32(b_emb),
        "ffn_w1t": ffn_w1t, "ffn_b1": f32(ffn_b1),
        "ffn_w2t": ffn_w2t, "ffn_b2": f32(ffn_b2),
        "g1": f32(g1), "be1": f32(be1), "g2": f32(g2), "be2": f32(be2),
    }

    roi_r = roi_feature.reshape(N_CORES, R, D, KK)
    img_r = image_feature.reshape(N_CORES, B, CIMG, S)
    in_maps = []
    for i in range(N_CORES):
        roi_t = np.ascontiguousarray(
            roi_r[i].transpose(1, 0, 2)).reshape(D, R * KK)
        in_maps.append({"roi_t": roi_t,
                        "img_raw": np.ascontiguousarray(img_r[i]),
                        **shared})
    return in_maps


def kernel(**inputs):
    global LAST_RESULTS
    in_maps = prepare_in_maps(**inputs)
    nc = _get_nc()
    trace = os.environ.get("BASS_KERNEL_TRACE", "0") == "1"
    if trace:
        _install_ntff_hook()
    LAST_RESULTS = run_bass_kernel_spmd(
        nc, in_maps, list(range(N_CORES)), trace=trace)
    out = np.concatenate([LAST_RESULTS.results[i]["out"]
                          for i in range(N_CORES)], axis=0)
    return out
